# revision 25
# baseline (speedup 1.0000x reference)
"""HGNN encoder (2-layer hypergraph message passing) as an 8-core TRN2 Bass kernel.

Strategy (1D node partition, K-sharded G-matmuls, bf16 streaming):
  - Each core owns a contiguous shard of user nodes (U/8) and item nodes (I/8).
  - All large tensors (incidence matrices, embeddings) are cast to bf16 on the
    host: halves HBM traffic and runs the PE at 1 cycle/row instead of 4
    (rel err ~2e-3, well inside the 2e-2 gate).
  - Layer matmuls user_hyper@user_emb / item_hyper@item_emb contract over
    nodes: each core multiplies its node-rows of the (host-pre-transposed,
    host-padded) incidence slices against its node shard of the embeddings,
    producing PARTIAL [64, G] messages in PSUM; per-part bf16 AllReduces
    (item part first, so its AR hides under the user stream) yield the full
    messages on every core.
  - DMA calls are batched aggressively (multiple 128-row tiles per dma_start
    via permuted DRAM access patterns): the sync engine pays a fixed ~0.5us
    per dma_start, so ~150 total DMAs instead of ~1200.
  - The attention/update block (tiny) is computed redundantly on every core in
    a transposed [64, G] layout (bf16 matmuls, fp32 PSUM), staged so each
    activation table (tanh/sigmoid) loads once per layer.
  - full_hyper@msg contracts over G in the transposed orientation
    (out^T = msg^T @ fh^T chunk): 16 wide 512-row matmuls per 512-node chunk
    with the msg tile stationary, then PE-transposes back to node-major.
    This is ~4x fewer PE instructions than the node-major orientation.
    fh chunks are prefetched before the AllReduce so the DMA pipe never
    idles; the residual stream (init emb + layer-1 out) is re-added lazily
    at the final drain from the bf16 residents.
"""

import numpy as np

U, I, G, D = 30000, 60000, 2000, 64
L = 2
NCORES = 8
UC, IC = U // NCORES, I // NCORES        # 3750, 7500
NKU = (UC + 127) // 128                  # 30 k-tiles (user shard)
NKI = (IC + 127) // 128                  # 59 k-tiles (item shard)
UCP, ICP = NKU * 128, NKI * 128          # host-padded shard sizes
GPAD = 2048
GT = GPAD // 128                         # 16 g-tiles
NCH = 512                                # phase-D node chunk (one PSUM bank wide)
AB = 4                                   # phase-A k-tiles batched per DMA


def _ch(n, step):
    return [(s, min(step, n - s)) for s in range(0, n, step)]


GCH = _ch(G, 512)                        # 4 free-dim chunks for matmul N<=512


def _build():
    import concourse.bacc as bacc
    import concourse.mybir as mybir
    import concourse.tile as tile
    from concourse import masks

    f32 = mybir.dt.float32
    bf16 = mybir.dt.bfloat16
    nc = bacc.Bacc("TRN2", target_bir_lowering=False, debug=False,
                   num_devices=NCORES)

    def din(name, shape, dt=f32):
        return nc.dram_tensor(name, shape, dt, kind="ExternalInput").ap()

    def dout(name, shape):
        return nc.dram_tensor(name, shape, f32, kind="ExternalOutput").ap()

    ue = din("ue", [UCP, D], bf16)
    ie = din("ie", [ICP, D], bf16)
    uhT = din("uhT", [UCP, G], bf16)
    ihT = din("ihT", [ICP, G], bf16)
    fhTu = din("fhTu", [GPAD, UC], bf16)
    fhTi = din("fhTi", [GPAD, IC], bf16)
    gT = din("gT", [D, G])
    qc_w1 = din("qc_w1", [L, D, D])
    qc_b1 = din("qc_b1", [L, D])
    qc_w2 = din("qc_w2", [L, D, 1])
    user_w = din("user_w", [L, 2 * D, D])
    user_b = din("user_b", [L, D])
    item_w = din("item_w", [L, 2 * D, D])
    item_b = din("item_b", [L, D])

    final_u = dout("final_u", [UC, D])
    final_i = dout("final_i", [IC, D])
    final_he = dout("final_he", [G, D])

    ujobs = [(0, n0, nw) for (n0, nw) in _ch(UC, NCH)]
    ijobs = [(1, n0, nw) for (n0, nw) in _ch(IC, NCH)]
    jobs = ujobs + ijobs
    PF = 3                               # fh chunks prefetched before the AR

    with tile.TileContext(nc) as tc:
        with (
            tc.tile_pool(name="const", bufs=1) as cpool,
            tc.tile_pool(name="resid", bufs=1) as rpool,
            tc.tile_pool(name="rhsA", bufs=3) as apool,
            tc.tile_pool(name="fhD", bufs=4) as dpool,
            tc.tile_pool(name="pb", bufs=2) as bpool,
            tc.tile_pool(name="outp", bufs=2) as opool,
            tc.tile_pool(name="psB", bufs=2, space="PSUM") as ps_b,
            tc.tile_pool(name="dram", bufs=1, space="DRAM") as drpool,
        ):
            # ---- constants / weights -------------------------------------
            ident = cpool.tile([128, 128], f32, tag="ident", name="ident")
            masks.make_identity(nc, ident[:])
            ones1b = cpool.tile([1, D], bf16, tag="ones1b", name="ones1b")
            nc.vector.memset(ones1b[:], 1.0)

            gTb = cpool.tile([D, GPAD], bf16, tag="gTb", name="gTb")
            nc.vector.memset(gTb[:, G:], 0.0)

            w1_s, w2_s, b1_s, uw_s, iw_s, bsum_s = [], [], [], [], [], []
            for l in range(L):
                stage = cpool.tile([2 * D, D], f32, tag=f"wst_{l}",
                                   name=f"wst_{l}")
                w1 = cpool.tile([D, D], bf16, tag=f"w1_{l}", name=f"w1_{l}")
                nc.sync.dma_start(stage[:D, :], qc_w1[l])
                nc.vector.tensor_copy(w1[:], stage[:D, :])
                w1_s.append(w1)
                w2f = cpool.tile([D, 1], f32, tag=f"w2f_{l}", name=f"w2f_{l}")
                nc.sync.dma_start(w2f[:], qc_w2[l])
                w2 = cpool.tile([D, 1], bf16, tag=f"w2_{l}", name=f"w2_{l}")
                nc.vector.tensor_copy(w2[:], w2f[:])
                w2n = cpool.tile([D, 1], bf16, tag=f"w2n_{l}", name=f"w2n_{l}")
                nc.vector.tensor_scalar_mul(w2n[:], w2f[:], -1.0)
                w2_s.append((w2, w2n))
                b1 = cpool.tile([D, 1], f32, tag=f"b1_{l}", name=f"b1_{l}")
                nc.sync.dma_start(b1[:], qc_b1[l].unsqueeze(1))
                b1_s.append(b1)
                st2 = cpool.tile([2 * D, D], f32, tag=f"wst2_{l}",
                                 name=f"wst2_{l}")
                nc.sync.dma_start(st2[:], user_w[l])
                uw0 = cpool.tile([D, D], bf16, tag=f"uw0_{l}", name=f"uw0_{l}")
                nc.vector.tensor_copy(uw0[:], st2[:D, :])
                uw1 = cpool.tile([D, D], bf16, tag=f"uw1_{l}", name=f"uw1_{l}")
                nc.vector.tensor_copy(uw1[:], st2[D:, :])
                uw_s.append((uw0, uw1))
                nc.sync.dma_start(stage[:], item_w[l])
                iw0 = cpool.tile([D, D], bf16, tag=f"iw0_{l}", name=f"iw0_{l}")
                nc.vector.tensor_copy(iw0[:], stage[:D, :])
                iw1 = cpool.tile([D, D], bf16, tag=f"iw1_{l}", name=f"iw1_{l}")
                nc.vector.tensor_copy(iw1[:], stage[D:, :])
                iw_s.append((iw0, iw1))
                ub = cpool.tile([D, 1], f32, tag=f"ub_{l}", name=f"ub_{l}")
                nc.sync.dma_start(ub[:], user_b[l].unsqueeze(1))
                ib = cpool.tile([D, 1], f32, tag=f"ib_{l}", name=f"ib_{l}")
                nc.sync.dma_start(ib[:], item_b[l].unsqueeze(1))
                bs = cpool.tile([D, 1], f32, tag=f"bs_{l}", name=f"bs_{l}")
                nc.vector.tensor_add(bs[:], ub[:], ib[:])
                bsum_s.append(bs)

            # ---- residents ----------------------------------------------
            ue_res = rpool.tile([128, NKU * D], bf16, tag="ue_res",
                                name="ue_res")
            ie_res = rpool.tile([128, NKI * D], bf16, tag="ie_res",
                                name="ie_res")
            ui_u = rpool.tile([128, NKU * D], bf16, tag="ui_u", name="ui_u")
            ui_i = rpool.tile([128, NKI * D], bf16, tag="ui_i", name="ui_i")
            he_acc = rpool.tile([D, GPAD], f32, tag="he_acc", name="he_acc")
            msgNT = rpool.tile([D, GPAD], f32, tag="msgNT", name="msgNT")
            mTu = rpool.tile([D, GPAD], bf16, tag="mTu", name="mTu")
            mTi = rpool.tile([D, GPAD], bf16, tag="mTi", name="mTi")
            msgP = rpool.tile([128, GT * D], bf16, tag="msgP", name="msgP")

            nc.sync.dma_start(
                ue_res[:].rearrange("p (t d) -> p t d", t=NKU),
                ue.rearrange("(t p) d -> p t d", p=128))
            nc.sync.dma_start(
                ie_res[:].rearrange("p (t d) -> p t d", t=NKI),
                ie.rearrange("(t p) d -> p t d", p=128))
            nc.vector.memset(ui_u[:], 0.0)
            nc.vector.memset(ui_i[:], 0.0)
            nc.sync.dma_start(he_acc[:, :G], gT[:, :])
            nc.vector.tensor_copy(gTb[:, :G], he_acc[:, :G])
            nc.vector.memset(msgNT[:, G:], 0.0)
            nc.vector.memset(mTu[:, G:], 0.0)
            nc.vector.memset(mTi[:, G:], 0.0)

            fhTu3 = fhTu.rearrange("(t p) n -> p t n", p=128)
            fhTi3 = fhTi.rearrange("(t p) n -> p t n", p=128)
            pf = {}                       # job idx -> prefetched fh tile

            def load_fh(j):
                part, n0, nw = jobs[j]
                src = fhTu3 if part == 0 else fhTi3
                ft = dpool.tile([128, GT * NCH], bf16, tag="fh",
                                name=f"fh_{j}")
                nc.sync.dma_start(
                    ft[:].rearrange("p (t n) -> p t n", n=NCH)[:, :, :nw],
                    src[:, :, n0:n0 + nw])
                pf[j] = ft

            for l in range(L):
                lhs_u = ue_res if l == 0 else ui_u
                lhs_i = ie_res if l == 0 else ui_i

                # ==== Phase A: partial messages, K-sharded over nodes ====
                # item part first; its (bigger) AllReduce hides under the
                # user stream, leaving only the small user AR exposed.
                with tc.tile_pool(name=f"psA{l}", bufs=1,
                                  space="PSUM") as ps_a:
                    for part, (hyT, nkt, lhs, mT) in enumerate([
                            (ihT, NKI, lhs_i, mTi), (uhT, NKU, lhs_u, mTu)]):
                        hyT3 = hyT.rearrange("(t p) g -> p t g", p=128)
                        ps_msg = ps_a.tile([64, GPAD], f32, tag="msgps",
                                           name=f"msgps_{l}_{part}")
                        for b0 in range(0, nkt, AB):
                            bb = min(AB, nkt - b0)
                            rt = apool.tile([128, AB * GPAD], bf16,
                                            tag="rhsA",
                                            name=f"rhsA_{l}_{part}_{b0}")
                            nc.sync.dma_start(
                                rt[:].rearrange("p (t g) -> p t g",
                                                g=GPAD)[:, :bb, :G],
                                hyT3[:, b0:b0 + bb, :])
                            for ti in range(bb):
                                t = b0 + ti
                                for (g0, gw) in GCH:
                                    nc.tensor.matmul(
                                        ps_msg[:, g0:g0 + gw],
                                        lhsT=lhs[:, t * D:(t + 1) * D],
                                        rhs=rt[:, ti * GPAD + g0:
                                               ti * GPAD + g0 + gw],
                                        start=(t == 0), stop=(t == nkt - 1))
                        pdrain = bpool.tile([64, G], bf16, tag="pdrain",
                                            bufs=1, name=f"pdrain_{l}_{part}")
                        nc.vector.tensor_copy(pdrain[:], ps_msg[:, :G])
                        cc_in = drpool.tile([64, G], bf16,
                                            tag=f"cc_in_{l}_{part}",
                                            name=f"cc_in_{l}_{part}")
                        cc_out = drpool.tile([64, G], bf16,
                                             tag=f"cc_out_{l}_{part}",
                                             addr_space="Shared",
                                             name=f"cc_out_{l}_{part}")
                        nc.sync.dma_start(cc_in[:], pdrain[:])
                        nc.gpsimd.collective_compute(
                            "AllReduce", mybir.AluOpType.add,
                            ins=[cc_in.opt()], outs=[cc_out.opt()],
                            replica_groups=[list(range(NCORES))])
                        nc.sync.dma_start(mT[:, :G], cc_out[:, :])
                        if part == 0:
                            # prefetch fh chunks under the exposed AR window
                            for j in range(PF):
                                load_fh(j)

                # ==== Phase B: attention + node-update weights (full G) ====
                # staged across G-chunks so each activation table loads once
                hus, his = [], []
                for ci, (g0, gw) in enumerate(GCH):
                    sl = slice(g0, g0 + gw)
                    hu_ps = ps_b.tile([64, 512], f32, tag="pb",
                                      name=f"hu_{l}_{ci}")
                    nc.tensor.matmul(hu_ps[:, :gw], lhsT=w1_s[l][:],
                                     rhs=mTu[:, sl], start=True, stop=True)
                    hu = bpool.tile([64, 512], bf16, tag="hu", bufs=4,
                                    name=f"hus_{l}_{ci}")
                    nc.scalar.activation(hu[:, :gw], hu_ps[:, :gw],
                                         mybir.ActivationFunctionType.Tanh,
                                         bias=b1_s[l][:])
                    hi_ps = ps_b.tile([64, 512], f32, tag="pb",
                                      name=f"hi_{l}_{ci}")
                    nc.tensor.matmul(hi_ps[:, :gw], lhsT=w1_s[l][:],
                                     rhs=mTi[:, sl], start=True, stop=True)
                    hi = bpool.tile([64, 512], bf16, tag="hi", bufs=4,
                                    name=f"his_{l}_{ci}")
                    nc.scalar.activation(hi[:, :gw], hi_ps[:, :gw],
                                         mybir.ActivationFunctionType.Tanh,
                                         bias=b1_s[l][:])
                    hus.append(hu)
                    his.append(hi)
                for ci, (g0, gw) in enumerate(GCH):
                    sl = slice(g0, g0 + gw)
                    um = mTu[:, sl]
                    im = mTi[:, sl]
                    # attention logit diff a_u - a_i accumulated in one bank;
                    # softmax over 2 == sigmoid(+-diff).  No other activation
                    # runs below, so the sigmoid table still loads only once.
                    ad_ps = ps_b.tile([1, 512], f32, tag="pb",
                                      name=f"ad_{l}_{ci}")
                    nc.tensor.matmul(ad_ps[:, :gw], lhsT=w2_s[l][0][:],
                                     rhs=hus[ci][:, :gw],
                                     start=True, stop=False)
                    nc.tensor.matmul(ad_ps[:, :gw], lhsT=w2_s[l][1][:],
                                     rhs=his[ci][:, :gw],
                                     start=False, stop=True)
                    wu = bpool.tile([1, 512], bf16, tag="wud", bufs=2,
                                    name=f"wu_{l}_{ci}")
                    nc.scalar.activation(wu[:, :gw], ad_ps[:, :gw],
                                         mybir.ActivationFunctionType.Sigmoid)
                    wi = bpool.tile([1, 512], bf16, tag="wud", bufs=2,
                                    name=f"wi_{l}_{ci}")
                    nc.scalar.activation(wi[:, :gw], ad_ps[:, :gw],
                                         mybir.ActivationFunctionType.Sigmoid,
                                         scale=-1.0)
                    # broadcast weights across 64 partitions via outer product
                    wub_ps = ps_b.tile([64, 512], f32, tag="pb",
                                       name=f"wub_{l}_{ci}")
                    nc.tensor.matmul(wub_ps[:, :gw], lhsT=ones1b[:],
                                     rhs=wu[:, :gw], start=True, stop=True)
                    wib_ps = ps_b.tile([64, 512], f32, tag="pb",
                                       name=f"wib_{l}_{ci}")
                    nc.tensor.matmul(wib_ps[:, :gw], lhsT=ones1b[:],
                                     rhs=wi[:, :gw], start=True, stop=True)
                    common = bpool.tile([64, 512], f32, tag="common",
                                        name=f"common_{l}_{ci}")
                    tmpc = bpool.tile([64, 512], f32, tag="tmpc",
                                      name=f"tmpc_{l}_{ci}")
                    nc.vector.tensor_mul(common[:, :gw], um, wub_ps[:, :gw])
                    nc.vector.tensor_mul(tmpc[:, :gw], im, wib_ps[:, :gw])
                    nc.vector.tensor_add(common[:, :gw], common[:, :gw],
                                         tmpc[:, :gw])
                    dfu = bpool.tile([64, 512], bf16, tag="dfu",
                                     name=f"dfu_{l}_{ci}")
                    dfi = bpool.tile([64, 512], bf16, tag="dfi",
                                     name=f"dfi_{l}_{ci}")
                    nc.vector.tensor_sub(dfu[:, :gw], um, common[:, :gw])
                    nc.vector.tensor_sub(dfi[:, :gw], im, common[:, :gw])
                    # u2+i2 accumulated: [diff,g] @ user_w + [diff,g] @ item_w
                    o2_ps = ps_b.tile([64, 512], f32, tag="pb",
                                      name=f"o2_{l}_{ci}")
                    nc.tensor.matmul(o2_ps[:, :gw], lhsT=uw_s[l][0][:],
                                     rhs=dfu[:, :gw], start=True, stop=False)
                    nc.tensor.matmul(o2_ps[:, :gw], lhsT=uw_s[l][1][:],
                                     rhs=gTb[:, sl], start=False, stop=False)
                    nc.tensor.matmul(o2_ps[:, :gw], lhsT=iw_s[l][0][:],
                                     rhs=dfi[:, :gw], start=False, stop=False)
                    nc.tensor.matmul(o2_ps[:, :gw], lhsT=iw_s[l][1][:],
                                     rhs=gTb[:, sl], start=False, stop=True)
                    # msg = u2 + i2 + (user_b+item_b) + common
                    nc.vector.scalar_tensor_tensor(
                        msgNT[:, sl], o2_ps[:, :gw], bsum_s[l][:],
                        common[:, :gw],
                        op0=mybir.AluOpType.add, op1=mybir.AluOpType.add)
                    nc.vector.tensor_add(he_acc[:, sl], he_acc[:, sl],
                                         msgNT[:, sl])
                    # Phase C interleaved: transpose this chunk's g-tiles to
                    # node-major now so Phase D can start accumulating while
                    # the remaining B chunks are still in flight.
                    for t in range(4 * ci, 4 * ci + 4):
                        tp_ps = ps_b.tile([128, 64], f32, tag="pb",
                                          name=f"tp_{l}_{t}")
                        nc.tensor.transpose(tp_ps[:],
                                            msgNT[:, t * 128:(t + 1) * 128],
                                            ident[:64, :64])
                        nc.vector.tensor_copy(msgP[:, t * D:(t + 1) * D],
                                              tp_ps[:])

                # ==== Phase D: node_outT = msgT @ full_hyperT, chunk-wise ====
                # out^T orientation: 16 wide matmuls per 512-node chunk
                # (lhsT = msgP g-tile), then PE-transpose back to node-major.
                with tc.tile_pool(name=f"psD{l}", bufs=2,
                                  space="PSUM") as ps_d:
                    for j, (part, n0, nw) in enumerate(jobs):
                        if j + PF < len(jobs):
                            load_fh(j + PF)
                        ft = pf.pop(j)
                        po = ps_d.tile([64, NCH], f32, tag="po",
                                       name=f"po_{l}_{j}")
                        for t in range(GT):
                            nc.tensor.matmul(
                                po[:, :nw],
                                lhsT=msgP[:, t * D:(t + 1) * D],
                                rhs=ft[:, t * NCH:t * NCH + nw],
                                start=(t == 0), stop=(t == GT - 1))
                        dnt = bpool.tile([64, NCH], f32, tag="dnt", bufs=2,
                                         name=f"dnt_{l}_{j}")
                        nc.vector.tensor_copy(dnt[:, :nw], po[:, :nw])
                        e_res = ue_res if part == 0 else ie_res
                        ui_res = ui_u if part == 0 else ui_i
                        fout = final_u if part == 0 else final_i
                        subs = _ch(nw, 128)
                        if l == 0:
                            for s, (s0, ss) in enumerate(subs):
                                ti = (n0 + s0) // 128
                                tsl = slice(ti * D, (ti + 1) * D)
                                tp = ps_d.tile([128, D], f32, tag="tp",
                                               name=f"tp_{l}_{j}_{s}")
                                nc.tensor.transpose(tp[:ss, :],
                                                    dnt[:, s0:s0 + ss],
                                                    ident[:64, :64])
                                nc.vector.tensor_copy(ui_res[:ss, tsl],
                                                      tp[:ss, :])
                        else:
                            fo = opool.tile([128, (NCH // 128) * D], f32,
                                            tag="fo", name=f"fo_{l}_{j}")
                            for s, (s0, ss) in enumerate(subs):
                                ti = (n0 + s0) // 128
                                tsl = slice(ti * D, (ti + 1) * D)
                                tp = ps_d.tile([128, D], f32, tag="tp",
                                               name=f"tp_{l}_{j}_{s}")
                                nc.tensor.transpose(tp[:ss, :],
                                                    dnt[:, s0:s0 + ss],
                                                    ident[:64, :64])
                                fsl = fo[:ss, s * D:(s + 1) * D]
                                nc.vector.tensor_add(fsl, e_res[:ss, tsl],
                                                     ui_res[:ss, tsl])
                                nc.vector.tensor_add(fsl, fsl, tp[:ss, :])
                            nf = sum(1 for (s0, ss) in subs if ss == 128)
                            if nf:
                                nc.sync.dma_start(
                                    fout[n0:n0 + nf * 128].rearrange(
                                        "(s p) d -> p s d", p=128),
                                    fo[:].rearrange("p (s d) -> p s d",
                                                    d=D)[:, :nf, :])
                            for s, (s0, ss) in enumerate(subs):
                                if ss != 128:
                                    nc.sync.dma_start(
                                        fout[n0 + s0:n0 + s0 + ss, :],
                                        fo[:ss, s * D:(s + 1) * D])

            # ==== final_he = group_emb + msg1 + msg2, transpose out ====
            ho = opool.tile([128, GT * D], f32, tag="ho", bufs=1, name="ho")
            for t, (g0, gg) in enumerate(_ch(G, 128)):
                tp_ps = ps_b.tile([128, 64], f32, tag="pb", name=f"he_t_{g0}")
                nc.tensor.transpose(tp_ps[:gg, :], he_acc[:, g0:g0 + gg],
                                    ident[:64, :64])
                nc.vector.tensor_copy(ho[:gg, t * D:(t + 1) * D],
                                      tp_ps[:gg, :])
            nfull = G // 128                  # 15 full tiles, tail of 80
            nc.sync.dma_start(
                final_he[:nfull * 128].rearrange("(s p) d -> p s d", p=128),
                ho[:].rearrange("p (s d) -> p s d", d=D)[:, :nfull, :])
            nc.sync.dma_start(final_he[nfull * 128:, :],
                              ho[:G - nfull * 128, nfull * D:nfull * D + D])

    nc.compile()
    return nc


_NC_CACHE = {}


def _get_nc():
    if "nc" not in _NC_CACHE:
        _NC_CACHE["nc"] = _build()
    return _NC_CACHE["nc"]


def _pad_rows(a, n):
    out = np.zeros((n,) + a.shape[1:], a.dtype)
    out[:a.shape[0]] = a
    return out


def make_in_maps(user_emb, item_emb, group_emb, user_hyper, item_hyper,
                 full_hyper, qc_w1, qc_b1, qc_w2, user_w, user_b, item_w,
                 item_b):
    import ml_dtypes
    bf = ml_dtypes.bfloat16
    f = np.float32
    rep = {
        "gT": np.ascontiguousarray(np.asarray(group_emb, f).T),
        "qc_w1": np.asarray(qc_w1, f), "qc_b1": np.asarray(qc_b1, f),
        "qc_w2": np.asarray(qc_w2, f),
        "user_w": np.asarray(user_w, f), "user_b": np.asarray(user_b, f),
        "item_w": np.asarray(item_w, f), "item_b": np.asarray(item_b, f),
    }
    ue_b = np.asarray(user_emb, f).astype(bf)
    ie_b = np.asarray(item_emb, f).astype(bf)
    uhT_all = np.ascontiguousarray(np.asarray(user_hyper, f).T.astype(bf))
    ihT_all = np.ascontiguousarray(np.asarray(item_hyper, f).T.astype(bf))
    fhT_all = np.ascontiguousarray(np.asarray(full_hyper, f).T.astype(bf))
    in_maps = []
    for c in range(NCORES):
        us = slice(c * UC, (c + 1) * UC)
        isl = slice(c * IC, (c + 1) * IC)
        m = dict(rep)
        m["ue"] = _pad_rows(ue_b[us], UCP)
        m["ie"] = _pad_rows(ie_b[isl], ICP)
        m["uhT"] = _pad_rows(uhT_all[us], UCP)
        m["ihT"] = _pad_rows(ihT_all[isl], ICP)
        m["fhTu"] = _pad_rows(np.ascontiguousarray(fhT_all[:, us]), GPAD)
        m["fhTi"] = _pad_rows(
            np.ascontiguousarray(fhT_all[:, U + c * IC:U + (c + 1) * IC]),
            GPAD)
        in_maps.append(m)
    return in_maps


def assemble(results):
    out = np.empty((U + I + G, D), np.float32)
    for c in range(NCORES):
        out[c * UC:(c + 1) * UC] = results[c]["final_u"]
        out[U + c * IC:U + (c + 1) * IC] = results[c]["final_i"]
    out[U + I:] = results[0]["final_he"]
    return out


def kernel(user_emb, item_emb, group_emb, user_hyper, item_hyper, full_hyper,
           qc_w1, qc_b1, qc_w2, user_w, user_b, item_w, item_b,
           num_users=U, num_items=I):
    from concourse.bass_utils import run_bass_kernel_spmd
    nc = _get_nc()
    in_maps = make_in_maps(user_emb, item_emb, group_emb, user_hyper,
                           item_hyper, full_hyper, qc_w1, qc_b1, qc_w2,
                           user_w, user_b, item_w, item_b)
    res = run_bass_kernel_spmd(nc, in_maps, list(range(NCORES)))
    return assemble(res.results)


# revision 27
# speedup vs baseline: 1.3074x; 1.3074x over previous
"""HGNN encoder (2-layer hypergraph message passing) as an 8-core TRN2 Bass kernel.

Strategy (1D node partition, K-sharded G-matmuls, bf16 streaming):
  - Each core owns a contiguous shard of user nodes (U/8) and item nodes (I/8).
  - All large tensors (incidence matrices, embeddings) are cast to bf16 on the
    host: halves HBM traffic and runs the PE at 1 cycle/row instead of 4
    (rel err ~2e-3, well inside the 2e-2 gate).
  - Layer matmuls user_hyper@user_emb / item_hyper@item_emb contract over
    nodes: each core multiplies its node-rows of the (host-pre-transposed,
    host-padded) incidence slices against its node shard of the embeddings,
    producing PARTIAL [64, G] messages in PSUM; per-part bf16 AllReduces
    (item part first, so its AR hides under the user stream) yield the full
    messages on every core.
  - DMA calls are batched aggressively (multiple 128-row tiles per dma_start
    via permuted DRAM access patterns): the sync engine pays a fixed ~0.5us
    per dma_start, so ~150 total DMAs instead of ~1200.
  - The attention/update block (tiny) is computed redundantly on every core in
    a transposed [64, G] layout (bf16 matmuls, fp32 PSUM), staged so each
    activation table (tanh/sigmoid) loads once per layer.
  - full_hyper@msg contracts over G in the transposed orientation
    (out^T = msg^T @ fh^T chunk): 16 wide 512-row matmuls per 512-node chunk
    with the msg tile stationary, then PE-transposes back to node-major.
    This is ~4x fewer PE instructions than the node-major orientation.
    fh chunks are prefetched before the AllReduce so the DMA pipe never
    idles; the residual stream (init emb + layer-1 out) is re-added lazily
    at the final drain from the bf16 residents.
"""

import numpy as np

U, I, G, D = 30000, 60000, 2000, 64
L = 2
NCORES = 8
UC, IC = U // NCORES, I // NCORES        # 3750, 7500
NKU = (UC + 127) // 128                  # 30 k-tiles (user shard)
NKI = (IC + 127) // 128                  # 59 k-tiles (item shard)
UCP, ICP = NKU * 128, NKI * 128          # host-padded shard sizes
GPAD = 2048
GT = GPAD // 128                         # 16 g-tiles
NCH = 512                                # phase-D node chunk (one PSUM bank wide)
AB = 4                                   # phase-A k-tiles batched per DMA


def _ch(n, step):
    return [(s, min(step, n - s)) for s in range(0, n, step)]


GCH = _ch(G, 512)                        # 4 free-dim chunks for matmul N<=512


def _build():
    import concourse.bacc as bacc
    import concourse.mybir as mybir
    import concourse.tile as tile
    from concourse import masks

    f32 = mybir.dt.float32
    bf16 = mybir.dt.bfloat16
    nc = bacc.Bacc("TRN2", target_bir_lowering=False, debug=False,
                   num_devices=NCORES)

    def din(name, shape, dt=f32):
        return nc.dram_tensor(name, shape, dt, kind="ExternalInput").ap()

    def dout(name, shape):
        return nc.dram_tensor(name, shape, f32, kind="ExternalOutput").ap()

    ue = din("ue", [UCP, D], bf16)
    ie = din("ie", [ICP, D], bf16)
    uhT = din("uhT", [UCP, G], bf16)
    ihT = din("ihT", [ICP, G], bf16)
    fhTu = din("fhTu", [GPAD, UC], bf16)
    fhTi = din("fhTi", [GPAD, IC], bf16)
    gT = din("gT", [D, G])
    qc_w1 = din("qc_w1", [L, D, D])
    qc_b1 = din("qc_b1", [L, D])
    qc_w2 = din("qc_w2", [L, D, 1])
    user_w = din("user_w", [L, 2 * D, D])
    user_b = din("user_b", [L, D])
    item_w = din("item_w", [L, 2 * D, D])
    item_b = din("item_b", [L, D])

    final_u = dout("final_u", [UC, D])
    final_i = dout("final_i", [IC, D])
    final_he = dout("final_he", [G, D])

    ujobs = [(0, n0, nw) for (n0, nw) in _ch(UC, NCH)]
    ijobs = [(1, n0, nw) for (n0, nw) in _ch(IC, NCH)]
    jobs = ujobs + ijobs
    PF = 3                               # fh chunks prefetched before the AR

    with tile.TileContext(nc) as tc:
        with (
            tc.tile_pool(name="const", bufs=1) as cpool,
            tc.tile_pool(name="resid", bufs=1) as rpool,
            tc.tile_pool(name="rhsA", bufs=3) as apool,
            tc.tile_pool(name="fhD", bufs=4) as dpool,
            tc.tile_pool(name="pb", bufs=2) as bpool,
            tc.tile_pool(name="outp", bufs=2) as opool,
            tc.tile_pool(name="psB", bufs=2, space="PSUM") as ps_b,
            tc.tile_pool(name="dram", bufs=1, space="DRAM") as drpool,
        ):
            # ---- constants / weights -------------------------------------
            ident = cpool.tile([128, 128], f32, tag="ident", name="ident")
            masks.make_identity(nc, ident[:])
            ones1b = cpool.tile([1, D], bf16, tag="ones1b", name="ones1b")
            nc.vector.memset(ones1b[:], 1.0)

            gTb = cpool.tile([D, GPAD], bf16, tag="gTb", name="gTb")
            nc.vector.memset(gTb[:, G:], 0.0)

            w1_s, w2_s, b1_s, uw_s, iw_s, bsum_s = [], [], [], [], [], []
            for l in range(L):
                stage = cpool.tile([2 * D, D], f32, tag=f"wst_{l}",
                                   name=f"wst_{l}")
                w1 = cpool.tile([D, D], bf16, tag=f"w1_{l}", name=f"w1_{l}")
                nc.sync.dma_start(stage[:D, :], qc_w1[l])
                nc.vector.tensor_copy(w1[:], stage[:D, :])
                w1_s.append(w1)
                w2f = cpool.tile([D, 1], f32, tag=f"w2f_{l}", name=f"w2f_{l}")
                nc.sync.dma_start(w2f[:], qc_w2[l])
                w2 = cpool.tile([D, 1], bf16, tag=f"w2_{l}", name=f"w2_{l}")
                nc.vector.tensor_copy(w2[:], w2f[:])
                w2n = cpool.tile([D, 1], bf16, tag=f"w2n_{l}", name=f"w2n_{l}")
                nc.vector.tensor_scalar_mul(w2n[:], w2f[:], -1.0)
                w2_s.append((w2, w2n))
                b1 = cpool.tile([D, 1], f32, tag=f"b1_{l}", name=f"b1_{l}")
                nc.sync.dma_start(b1[:], qc_b1[l].unsqueeze(1))
                b1_s.append(b1)
                st2 = cpool.tile([2 * D, D], f32, tag=f"wst2_{l}",
                                 name=f"wst2_{l}")
                nc.sync.dma_start(st2[:], user_w[l])
                uw0 = cpool.tile([D, D], bf16, tag=f"uw0_{l}", name=f"uw0_{l}")
                nc.vector.tensor_copy(uw0[:], st2[:D, :])
                uw1 = cpool.tile([D, D], bf16, tag=f"uw1_{l}", name=f"uw1_{l}")
                nc.vector.tensor_copy(uw1[:], st2[D:, :])
                uw_s.append((uw0, uw1))
                nc.sync.dma_start(stage[:], item_w[l])
                iw0 = cpool.tile([D, D], bf16, tag=f"iw0_{l}", name=f"iw0_{l}")
                nc.vector.tensor_copy(iw0[:], stage[:D, :])
                iw1 = cpool.tile([D, D], bf16, tag=f"iw1_{l}", name=f"iw1_{l}")
                nc.vector.tensor_copy(iw1[:], stage[D:, :])
                iw_s.append((iw0, iw1))
                ub = cpool.tile([D, 1], f32, tag=f"ub_{l}", name=f"ub_{l}")
                nc.sync.dma_start(ub[:], user_b[l].unsqueeze(1))
                ib = cpool.tile([D, 1], f32, tag=f"ib_{l}", name=f"ib_{l}")
                nc.sync.dma_start(ib[:], item_b[l].unsqueeze(1))
                bs = cpool.tile([D, 1], f32, tag=f"bs_{l}", name=f"bs_{l}")
                nc.vector.tensor_add(bs[:], ub[:], ib[:])
                bsum_s.append(bs)

            # ---- residents ----------------------------------------------
            ue_res = rpool.tile([128, NKU * D], bf16, tag="ue_res",
                                name="ue_res")
            ie_res = rpool.tile([128, NKI * D], bf16, tag="ie_res",
                                name="ie_res")
            ui_u = rpool.tile([128, NKU * D], bf16, tag="ui_u", name="ui_u")
            ui_i = rpool.tile([128, NKI * D], bf16, tag="ui_i", name="ui_i")
            he_acc = rpool.tile([D, GPAD], f32, tag="he_acc", name="he_acc")
            msgNT = rpool.tile([D, GPAD], f32, tag="msgNT", name="msgNT")
            mTu = rpool.tile([D, GPAD], bf16, tag="mTu", name="mTu")
            mTi = rpool.tile([D, GPAD], bf16, tag="mTi", name="mTi")
            msgP = rpool.tile([128, GT * D], bf16, tag="msgP", name="msgP")

            nc.sync.dma_start(
                ue_res[:].rearrange("p (t d) -> p t d", t=NKU),
                ue.rearrange("(t p) d -> p t d", p=128))
            nc.sync.dma_start(
                ie_res[:].rearrange("p (t d) -> p t d", t=NKI),
                ie.rearrange("(t p) d -> p t d", p=128))
            nc.vector.memset(ui_u[:], 0.0)
            nc.vector.memset(ui_i[:], 0.0)
            nc.sync.dma_start(he_acc[:, :G], gT[:, :])
            nc.vector.tensor_copy(gTb[:, :G], he_acc[:, :G])
            nc.vector.memset(msgNT[:, G:], 0.0)
            nc.vector.memset(mTu[:, G:], 0.0)
            nc.vector.memset(mTi[:, G:], 0.0)

            fhTu3 = fhTu.rearrange("(t p) n -> p t n", p=128)
            fhTi3 = fhTi.rearrange("(t p) n -> p t n", p=128)
            pf = {}                       # job idx -> prefetched fh tile

            def load_fh(j):
                part, n0, nw = jobs[j]
                src = fhTu3 if part == 0 else fhTi3
                ft = dpool.tile([128, GT * NCH], bf16, tag="fh",
                                name=f"fh_{j}")
                nc.sync.dma_start(
                    ft[:].rearrange("p (t n) -> p t n", n=NCH)[:, :, :nw],
                    src[:, :, n0:n0 + nw])
                pf[j] = ft

            for l in range(L):
                lhs_u = ue_res if l == 0 else ui_u
                lhs_i = ie_res if l == 0 else ui_i

                # ==== Phase A: partial messages, K-sharded over nodes ====
                # item part first; its (bigger) AllReduce hides under the
                # user stream, leaving only the small user AR exposed.
                with tc.tile_pool(name=f"psA{l}", bufs=1,
                                  space="PSUM") as ps_a:
                    for part, (hyT, nkt, lhs, mT) in enumerate([
                            (ihT, NKI, lhs_i, mTi), (uhT, NKU, lhs_u, mTu)]):
                        hyT3 = hyT.rearrange("(t p) g -> p t g", p=128)
                        ps_msg = ps_a.tile([64, GPAD], f32, tag="msgps",
                                           name=f"msgps_{l}_{part}")
                        for b0 in range(0, nkt, AB):
                            bb = min(AB, nkt - b0)
                            rt = apool.tile([128, AB * GPAD], bf16,
                                            tag="rhsA",
                                            name=f"rhsA_{l}_{part}_{b0}")
                            nc.sync.dma_start(
                                rt[:].rearrange("p (t g) -> p t g",
                                                g=GPAD)[:, :bb, :G],
                                hyT3[:, b0:b0 + bb, :])
                            for ti in range(bb):
                                t = b0 + ti
                                for (g0, gw) in GCH:
                                    nc.tensor.matmul(
                                        ps_msg[:, g0:g0 + gw],
                                        lhsT=lhs[:, t * D:(t + 1) * D],
                                        rhs=rt[:, ti * GPAD + g0:
                                               ti * GPAD + g0 + gw],
                                        start=(t == 0), stop=(t == nkt - 1))
                        pdrain = bpool.tile([64, G], bf16, tag="pdrain",
                                            bufs=1, name=f"pdrain_{l}_{part}")
                        nc.vector.tensor_copy(pdrain[:], ps_msg[:, :G])
                        cc_in = drpool.tile([64, G], bf16,
                                            tag=f"cc_in_{l}_{part}",
                                            name=f"cc_in_{l}_{part}")
                        cc_out = drpool.tile([64, G], bf16,
                                             tag=f"cc_out_{l}_{part}",
                                             addr_space="Shared",
                                             name=f"cc_out_{l}_{part}")
                        nc.sync.dma_start(cc_in[:], pdrain[:])
                        nc.gpsimd.collective_compute(
                            "AllReduce", mybir.AluOpType.add,
                            ins=[cc_in.opt()], outs=[cc_out.opt()],
                            replica_groups=[list(range(NCORES))])
                        nc.sync.dma_start(mT[:, :G], cc_out[:, :])
                        if part == 0:
                            # prefetch fh chunks under the exposed AR window
                            for j in range(PF):
                                load_fh(j)

                # ==== Phase B: attention + node-update weights (full G) ====
                # staged across G-chunks so each activation table loads once
                hus, his = [], []
                for ci, (g0, gw) in enumerate(GCH):
                    sl = slice(g0, g0 + gw)
                    hu_ps = ps_b.tile([64, 512], f32, tag="pb",
                                      name=f"hu_{l}_{ci}")
                    nc.tensor.matmul(hu_ps[:, :gw], lhsT=w1_s[l][:],
                                     rhs=mTu[:, sl], start=True, stop=True)
                    hu = bpool.tile([64, 512], bf16, tag="hu", bufs=4,
                                    name=f"hus_{l}_{ci}")
                    nc.scalar.activation(hu[:, :gw], hu_ps[:, :gw],
                                         mybir.ActivationFunctionType.Tanh,
                                         bias=b1_s[l][:])
                    hi_ps = ps_b.tile([64, 512], f32, tag="pb",
                                      name=f"hi_{l}_{ci}")
                    nc.tensor.matmul(hi_ps[:, :gw], lhsT=w1_s[l][:],
                                     rhs=mTi[:, sl], start=True, stop=True)
                    hi = bpool.tile([64, 512], bf16, tag="hi", bufs=4,
                                    name=f"his_{l}_{ci}")
                    nc.scalar.activation(hi[:, :gw], hi_ps[:, :gw],
                                         mybir.ActivationFunctionType.Tanh,
                                         bias=b1_s[l][:])
                    hus.append(hu)
                    his.append(hi)
                for ci, (g0, gw) in enumerate(GCH):
                    sl = slice(g0, g0 + gw)
                    um = mTu[:, sl]
                    im = mTi[:, sl]
                    # attention logit diff a_u - a_i accumulated in one bank;
                    # softmax over 2 == sigmoid(+-diff).  No other activation
                    # runs below, so the sigmoid table still loads only once.
                    ad_ps = ps_b.tile([1, 512], f32, tag="pb",
                                      name=f"ad_{l}_{ci}")
                    nc.tensor.matmul(ad_ps[:, :gw], lhsT=w2_s[l][0][:],
                                     rhs=hus[ci][:, :gw],
                                     start=True, stop=False)
                    nc.tensor.matmul(ad_ps[:, :gw], lhsT=w2_s[l][1][:],
                                     rhs=his[ci][:, :gw],
                                     start=False, stop=True)
                    wu = bpool.tile([1, 512], bf16, tag="wud", bufs=2,
                                    name=f"wu_{l}_{ci}")
                    nc.scalar.activation(wu[:, :gw], ad_ps[:, :gw],
                                         mybir.ActivationFunctionType.Sigmoid)
                    wi = bpool.tile([1, 512], bf16, tag="wud", bufs=2,
                                    name=f"wi_{l}_{ci}")
                    nc.scalar.activation(wi[:, :gw], ad_ps[:, :gw],
                                         mybir.ActivationFunctionType.Sigmoid,
                                         scale=-1.0)
                    # broadcast weights across 64 partitions via outer product
                    wub_ps = ps_b.tile([64, 512], f32, tag="pb",
                                       name=f"wub_{l}_{ci}")
                    nc.tensor.matmul(wub_ps[:, :gw], lhsT=ones1b[:],
                                     rhs=wu[:, :gw], start=True, stop=True)
                    wib_ps = ps_b.tile([64, 512], f32, tag="pb",
                                       name=f"wib_{l}_{ci}")
                    nc.tensor.matmul(wib_ps[:, :gw], lhsT=ones1b[:],
                                     rhs=wi[:, :gw], start=True, stop=True)
                    common = bpool.tile([64, 512], f32, tag="common",
                                        name=f"common_{l}_{ci}")
                    tmpc = bpool.tile([64, 512], f32, tag="tmpc",
                                      name=f"tmpc_{l}_{ci}")
                    nc.vector.tensor_mul(common[:, :gw], um, wub_ps[:, :gw])
                    nc.vector.tensor_mul(tmpc[:, :gw], im, wib_ps[:, :gw])
                    nc.vector.tensor_add(common[:, :gw], common[:, :gw],
                                         tmpc[:, :gw])
                    dfu = bpool.tile([64, 512], bf16, tag="dfu",
                                     name=f"dfu_{l}_{ci}")
                    dfi = bpool.tile([64, 512], bf16, tag="dfi",
                                     name=f"dfi_{l}_{ci}")
                    nc.vector.tensor_sub(dfu[:, :gw], um, common[:, :gw])
                    nc.vector.tensor_sub(dfi[:, :gw], im, common[:, :gw])
                    # u2+i2 accumulated: [diff,g] @ user_w + [diff,g] @ item_w
                    o2_ps = ps_b.tile([64, 512], f32, tag="pb",
                                      name=f"o2_{l}_{ci}")
                    nc.tensor.matmul(o2_ps[:, :gw], lhsT=uw_s[l][0][:],
                                     rhs=dfu[:, :gw], start=True, stop=False)
                    nc.tensor.matmul(o2_ps[:, :gw], lhsT=uw_s[l][1][:],
                                     rhs=gTb[:, sl], start=False, stop=False)
                    nc.tensor.matmul(o2_ps[:, :gw], lhsT=iw_s[l][0][:],
                                     rhs=dfi[:, :gw], start=False, stop=False)
                    nc.tensor.matmul(o2_ps[:, :gw], lhsT=iw_s[l][1][:],
                                     rhs=gTb[:, sl], start=False, stop=True)
                    # msg = u2 + i2 + (user_b+item_b) + common
                    nc.vector.scalar_tensor_tensor(
                        msgNT[:, sl], o2_ps[:, :gw], bsum_s[l][:],
                        common[:, :gw],
                        op0=mybir.AluOpType.add, op1=mybir.AluOpType.add)
                    nc.vector.tensor_add(he_acc[:, sl], he_acc[:, sl],
                                         msgNT[:, sl])
                    # Phase C interleaved: transpose this chunk's g-tiles to
                    # node-major now so Phase D can start accumulating while
                    # the remaining B chunks are still in flight.
                    for t in range(4 * ci, 4 * ci + 4):
                        tp_ps = ps_b.tile([128, 64], f32, tag="pb",
                                          name=f"tp_{l}_{t}")
                        nc.tensor.transpose(tp_ps[:],
                                            msgNT[:, t * 128:(t + 1) * 128],
                                            ident[:64, :64])
                        nc.vector.tensor_copy(msgP[:, t * D:(t + 1) * D],
                                              tp_ps[:])

                # ==== Phase D: node_outT = msgT @ full_hyperT, chunk-wise ====
                # out^T orientation: 16 wide matmuls per 512-node chunk
                # (lhsT = msgP g-tile), then PE-transpose back to node-major.
                with tc.tile_pool(name=f"psD{l}", bufs=2,
                                  space="PSUM") as ps_d:
                    for j, (part, n0, nw) in enumerate(jobs):
                        if j + PF < len(jobs):
                            load_fh(j + PF)
                        ft = pf.pop(j)
                        po = ps_d.tile([64, NCH], f32, tag="po",
                                       name=f"po_{l}_{j}")
                        for t in range(GT):
                            nc.tensor.matmul(
                                po[:, :nw],
                                lhsT=msgP[:, t * D:(t + 1) * D],
                                rhs=ft[:, t * NCH:t * NCH + nw],
                                start=(t == 0), stop=(t == GT - 1))
                        dnt = bpool.tile([64, NCH], f32, tag="dnt", bufs=2,
                                         name=f"dnt_{l}_{j}")
                        nc.vector.tensor_copy(dnt[:, :nw], po[:, :nw])
                        e_res = ue_res if part == 0 else ie_res
                        ui_res = ui_u if part == 0 else ui_i
                        fout = final_u if part == 0 else final_i
                        subs = _ch(nw, 128)
                        if l == 0:
                            for s, (s0, ss) in enumerate(subs):
                                ti = (n0 + s0) // 128
                                tsl = slice(ti * D, (ti + 1) * D)
                                tp = ps_d.tile([128, D], f32, tag="tp",
                                               name=f"tp_{l}_{j}_{s}")
                                nc.tensor.transpose(tp[:ss, :],
                                                    dnt[:, s0:s0 + ss],
                                                    ident[:64, :64])
                                nc.vector.tensor_copy(ui_res[:ss, tsl],
                                                      tp[:ss, :])
                        else:
                            fo = opool.tile([128, (NCH // 128) * D], f32,
                                            tag="fo", name=f"fo_{l}_{j}")
                            for s, (s0, ss) in enumerate(subs):
                                ti = (n0 + s0) // 128
                                tsl = slice(ti * D, (ti + 1) * D)
                                tp = ps_d.tile([128, D], f32, tag="tp",
                                               name=f"tp_{l}_{j}_{s}")
                                nc.tensor.transpose(tp[:ss, :],
                                                    dnt[:, s0:s0 + ss],
                                                    ident[:64, :64])
                                fsl = fo[:ss, s * D:(s + 1) * D]
                                nc.vector.tensor_add(fsl, e_res[:ss, tsl],
                                                     ui_res[:ss, tsl])
                                nc.vector.tensor_add(fsl, fsl, tp[:ss, :])
                            nf = sum(1 for (s0, ss) in subs if ss == 128)
                            if nf:
                                nc.sync.dma_start(
                                    fout[n0:n0 + nf * 128].rearrange(
                                        "(s p) d -> p s d", p=128),
                                    fo[:].rearrange("p (s d) -> p s d",
                                                    d=D)[:, :nf, :])
                            for s, (s0, ss) in enumerate(subs):
                                if ss != 128:
                                    nc.sync.dma_start(
                                        fout[n0 + s0:n0 + s0 + ss, :],
                                        fo[:ss, s * D:(s + 1) * D])

            # ==== final_he = group_emb + msg1 + msg2, transpose out ====
            ho = opool.tile([128, GT * D], f32, tag="ho", bufs=1, name="ho")
            for t, (g0, gg) in enumerate(_ch(G, 128)):
                tp_ps = ps_b.tile([128, 64], f32, tag="pb", name=f"he_t_{g0}")
                nc.tensor.transpose(tp_ps[:gg, :], he_acc[:, g0:g0 + gg],
                                    ident[:64, :64])
                nc.vector.tensor_copy(ho[:gg, t * D:(t + 1) * D],
                                      tp_ps[:gg, :])
            nfull = G // 128                  # 15 full tiles, tail of 80
            nc.sync.dma_start(
                final_he[:nfull * 128].rearrange("(s p) d -> p s d", p=128),
                ho[:].rearrange("p (s d) -> p s d", d=D)[:, :nfull, :])
            nc.sync.dma_start(final_he[nfull * 128:, :],
                              ho[:G - nfull * 128, nfull * D:nfull * D + D])

    nc.compile()
    return nc


_NC_CACHE = {}


def _get_nc():
    if "nc" not in _NC_CACHE:
        _NC_CACHE["nc"] = _build()
    return _NC_CACHE["nc"]


def _pad_rows(a, n):
    out = np.zeros((n,) + a.shape[1:], a.dtype)
    out[:a.shape[0]] = a
    return out


def make_in_maps(user_emb, item_emb, group_emb, user_hyper, item_hyper,
                 full_hyper, qc_w1, qc_b1, qc_w2, user_w, user_b, item_w,
                 item_b):
    import ml_dtypes
    bf = ml_dtypes.bfloat16
    f = np.float32
    rep = {
        "gT": np.ascontiguousarray(np.asarray(group_emb, f).T),
        "qc_w1": np.asarray(qc_w1, f), "qc_b1": np.asarray(qc_b1, f),
        "qc_w2": np.asarray(qc_w2, f),
        "user_w": np.asarray(user_w, f), "user_b": np.asarray(user_b, f),
        "item_w": np.asarray(item_w, f), "item_b": np.asarray(item_b, f),
    }
    ue_b = np.asarray(user_emb, f).astype(bf)
    ie_b = np.asarray(item_emb, f).astype(bf)
    uhT_all = np.ascontiguousarray(np.asarray(user_hyper, f).T.astype(bf))
    ihT_all = np.ascontiguousarray(np.asarray(item_hyper, f).T.astype(bf))
    fhT_all = np.ascontiguousarray(np.asarray(full_hyper, f).T.astype(bf))
    in_maps = []
    for c in range(NCORES):
        us = slice(c * UC, (c + 1) * UC)
        isl = slice(c * IC, (c + 1) * IC)
        m = dict(rep)
        m["ue"] = _pad_rows(ue_b[us], UCP)
        m["ie"] = _pad_rows(ie_b[isl], ICP)
        m["uhT"] = _pad_rows(uhT_all[us], UCP)
        m["ihT"] = _pad_rows(ihT_all[isl], ICP)
        m["fhTu"] = _pad_rows(np.ascontiguousarray(fhT_all[:, us]), GPAD)
        m["fhTi"] = _pad_rows(
            np.ascontiguousarray(fhT_all[:, U + c * IC:U + (c + 1) * IC]),
            GPAD)
        in_maps.append(m)
    return in_maps


def assemble(results):
    out = np.empty((U + I + G, D), np.float32)
    for c in range(NCORES):
        out[c * UC:(c + 1) * UC] = results[c]["final_u"]
        out[U + c * IC:U + (c + 1) * IC] = results[c]["final_i"]
    out[U + I:] = results[0]["final_he"]
    return out


def kernel(user_emb, item_emb, group_emb, user_hyper, item_hyper, full_hyper,
           qc_w1, qc_b1, qc_w2, user_w, user_b, item_w, item_b,
           num_users=U, num_items=I):
    from concourse.bass_utils import run_bass_kernel_spmd
    nc = _get_nc()
    in_maps = make_in_maps(user_emb, item_emb, group_emb, user_hyper,
                           item_hyper, full_hyper, qc_w1, qc_b1, qc_w2,
                           user_w, user_b, item_w, item_b)
    res = run_bass_kernel_spmd(nc, in_maps, list(range(NCORES)))
    return assemble(res.results)
